# revision 27
# baseline (speedup 1.0000x reference)
"""Trainium2 Bass kernel: AssignmentLSTMLanguageModel.

Model: per-token 2-way LSTM cell assignment, then [B*T, V] log_softmax
projection.  B=16, T=256, E=256, H=512, V=32000, 8 NeuronCores.

Strategy
--------
- LSTM is replicated on all 8 cores (sequential recurrence; collectives have
  ~10-20us latency floors so per-step sharding is impossible).
- The input-side projection Xsel[b,t] = emb @ W_ih[cid].T + b[cid] has no
  recurrent dependency, so it is computed in an efficient batched phase
  (128-row t-major tiles) interleaved with the LSTM via an SBUF ring buffer.
- Per step only the recurrent part runs: 8 K-tile matmuls per gate strip
  (h @ 0.5*W1hh.T accumulated with (h*m0) @ 0.5*dWhh.T -- the per-token cell
  *selection* done algebraically) plus one K=16 identity-inject matmul that
  adds the precomputed Xsel row.  Weights are the moving operand in float32r
  (1 cycle/row).
- sigmoid(x) is computed as 0.5*(tanh(x/2)+1) so the whole inner loop uses
  only the Tanh activation table (no ACT table-switch stalls); the 0.5/2
  factors fold into doubled state (c'=2c, h'=2h) and host-prescaled weights.
- Activated gates are PE-transposed to [hidden, batch] layout so the LSTM
  cell elementwise chain runs on all 128 lanes, and h'.T lands directly in
  the big hs.T buffer that phase 2 consumes as stationary operands.
- Projection/log_softmax is vocab-sharded: each core owns a V/8 slice of
  W_out in SBUF, pass 1 computes sum(exp(z)) per row (no max subtraction
  needed: |z|~1 by construction), one small AllGather combines partial sums,
  pass 2 recomputes z and writes z - log(sum) straight to HBM.  The 524MB
  logits tensor is written exactly once.
- Host side (inside kernel()): embedding gather, layout transposes, masks,
  weight prescaling, output concat.
"""

import sys

sys.path.insert(0, "/opt/trn_rl_repo")

import numpy as np

import concourse.bass as bass
import concourse.mybir as mybir
import concourse.tile as tile
from concourse import bacc
from concourse.bass_utils import run_bass_kernel_spmd


def _install_ntff_shim():
    """The image's antenv package lacks axon_hooks, so trace=True silently
    degrades.  Provide the module and register the ctypes-based NTFF hook so
    run_bass_kernel_spmd(trace=True) returns exec_time_ns."""
    import types

    if "antenv.axon_hooks" in sys.modules:
        return
    mod = types.ModuleType("antenv.axon_hooks")
    mod._hook = None

    def set_axon_ntff_profile_hook(h):
        mod._hook = h

    def get_axon_ntff_profile_hook():
        return mod._hook

    mod.set_axon_ntff_profile_hook = set_axon_ntff_profile_hook
    mod.get_axon_ntff_profile_hook = get_axon_ntff_profile_hook
    sys.modules["antenv.axon_hooks"] = mod
    try:
        from trn_agent_boot.trn_boot import _ntff_profile_via_ctypes

        hook = _ntff_profile_via_ctypes("/opt/axon/libaxon_pjrt.so")
        if hook is not None:
            mod._hook = hook
    except Exception:
        pass


_install_ntff_shim()

F32 = mybir.dt.float32
F32R = mybir.dt.float32r

B = 16
E = 256
H = 512
NJ = H // 128  # 4 k-tiles over hidden
NG = 4  # gates i,f,g,o
GW = NG * H  # 2048 total gate width
XLEAD = 1  # xsel ring lead (row tiles)


def _f(ap):
    """view a float32r AP as plain f32 for vector/scalar engines"""
    if ap.dtype == F32R:
        ap = ap.bitcast(F32)
    return ap


def build_graph(nc, T, V, n_cores, has_bout, chunk=500):
    VS = V // n_cores
    ROWS = B * T
    assert ROWS % 128 == 0
    R = ROWS // 128  # row tiles (t-major: row = 16*t + b)
    SPT = 128 // B  # steps per row tile (8)
    chunks = []
    v = 0
    while v < VS:
        chunks.append((v, min(chunk, VS - v)))
        v += chunk
    NCH = len(chunks)

    # ---------------- DRAM I/O ----------------
    SC = nc.dram_tensor("SC", [T, 128, 64], F32, kind="ExternalInput").ap()
    RHS_H = nc.dram_tensor("RHS_H", [128, 2 * NJ, GW], F32R, kind="ExternalInput").ap()
    RHS_E = nc.dram_tensor("RHS_E", [128, 4, GW], F32R, kind="ExternalInput").ap()
    RHS_B = nc.dram_tensor("RHS_B", [32, GW], F32R, kind="ExternalInput").ap()
    EMB_T = nc.dram_tensor("EMB_T", [2, 128, ROWS], F32R, kind="ExternalInput").ap()
    EMB0_T = nc.dram_tensor("EMB0_T", [2, 128, ROWS], F32R, kind="ExternalInput").ap()
    BB = nc.dram_tensor("BB", [R, 32, 128], F32R, kind="ExternalInput").ap()
    CONSTS = nc.dram_tensor("CONSTS", [128, 209], F32, kind="ExternalInput").ap()
    I16R = nc.dram_tensor("I16R", [16, 16], F32R, kind="ExternalInput").ap()
    WOUT = nc.dram_tensor("WOUT", [128, NJ, VS], F32R, kind="ExternalInput").ap()
    if has_bout:
        BOUT = nc.dram_tensor("BOUT", [128, VS], F32, kind="ExternalInput").ap()
    OUT = nc.dram_tensor("OUT", [ROWS, VS], F32, kind="ExternalOutput").ap()
    HT2 = nc.dram_tensor("HT2", [128, NJ * B], F32R, kind="ExternalOutput").ap()
    CT2 = nc.dram_tensor("CT2", [128, NJ * B], F32, kind="ExternalOutput").ap()

    with tile.TileContext(nc) as tc:
        import contextlib

        ctx = contextlib.ExitStack()
        with ctx:
            # ---------------- SBUF ----------------
            st = ctx.enter_context(tc.tile_pool(name="statics", bufs=1))

            def static(shape, name, dtype=F32):
                return st.tile(shape, dtype, name=name, tag=name)

            # LSTM h-weights; phase 2 reuses the same bytes for W_out slice
            bigbuf = static([128, 2 * NJ * GW], "bigbuf", F32R)
            rhs_h = bigbuf.rearrange("p (a n) -> p a n", a=2 * NJ)
            wout_sb = bigbuf[:, : NJ * VS].rearrange("p (a n) -> p a n", a=NJ)
            rhs_e = static([128, 4, GW], "rhs_e", F32R)
            rhs_b = static([32, GW], "rhs_b", F32R)
            consts = static([128, 209], "consts")
            i16r = static([16, 16], "i16r", F32R)
            hsT = static([128, NJ, T + 1, B], "hsT", F32R)
            cprime = static([128, NJ * B], "cprime")
            h0buf = static([128, NJ * B], "h0buf", F32R)
            tg = static([16, GW], "tg")
            tmp_u = static([128, NJ * B], "tmp_u")
            tmp_v = static([128, NJ * B], "tmp_v")
            tanh_c = static([128, NJ * B], "tanh_c")
            gg_sb = static([128, NJ * B], "gg_sb")
            if has_bout:
                bout_sb = static([128, VS], "bout_sb")
            sparts = static([128, R, NCH], "sparts")
            s_loc = static([128, R], "s_loc")
            s_all = static([128, R, n_cores], "s_all")
            lse = static([128, R], "lse")

            sc_pool = ctx.enter_context(tc.tile_pool(name="sc_pool", bufs=3))
            ebuf_pool = ctx.enter_context(tc.tile_pool(name="ebuf_pool", bufs=2))
            bb_pool = ctx.enter_context(tc.tile_pool(name="bb_pool", bufs=2))
            xsel_pool = ctx.enter_context(
                tc.tile_pool(name="xsel_pool", bufs=XLEAD + 1)
            )
            xstep_pool = ctx.enter_context(tc.tile_pool(name="xstep_pool", bufs=1))
            if has_bout:
                scr_pool = ctx.enter_context(tc.tile_pool(name="scr_pool", bufs=3))
            outb_pool = ctx.enter_context(tc.tile_pool(name="outb_pool", bufs=3))

            # ---------------- PSUM (8 banks total) ----------------
            psum_pool = ctx.enter_context(
                tc.tile_pool(name="psum", bufs=1, space="PSUM")
            )
            # gates: 4 banks; strip s at [0:16, 512s:512s+512]
            gates_ps = psum_pool.tile([128, GW], F32, name="gates_ps", tag="gates_ps")
            tgps = psum_pool.tile([128, NG * NJ * B], F32, name="tgps", tag="tgps")
            acc_pool = ctx.enter_context(
                tc.tile_pool(name="acc_pool", bufs=3, space="PSUM")
            )

            dram_pool = ctx.enter_context(
                tc.tile_pool(name="dram", bufs=1, space="DRAM")
            )
            cc_in = dram_pool.tile([128, R], F32, name="cc_in", tag="cc_in")
            cc_out = dram_pool.tile(
                [n_cores, 128, R], F32, name="cc_out", tag="cc_out",
                addr_space="Shared" if n_cores > 4 else "Local",
            )

            # ---------------- init DMAs ----------------
            nc.sync.dma_start(out=consts, in_=CONSTS)
            nc.sync.dma_start(out=i16r, in_=I16R)
            nc.sync.dma_start(
                out=hsT[:, :, 0, :],
                in_=CONSTS[:, 0:64].rearrange("p (a b) -> p a b", a=NJ).bitcast(F32R),
            )
            nc.sync.dma_start(out=h0buf, in_=CONSTS[:, 64:128].bitcast(F32R))
            nc.sync.dma_start(out=cprime, in_=CONSTS[:, 128:192])
            nc.sync.dma_start(
                out=bigbuf.rearrange("p (a n) -> p a n", a=2 * NJ), in_=RHS_H
            )
            nc.sync.dma_start(out=rhs_e, in_=RHS_E)
            nc.sync.dma_start(out=rhs_b, in_=RHS_B)

            idmat = consts[:, 192:208]  # [128,16]; rows 0:16 used

            # ---------------- batched Xsel phase (ring) ----------------
            xsel_tiles = {}

            def xsel_rowtile(r):
                """Xsel rows [128r:128r+128] (t-major) -> SBUF ring tile."""
                eb = ebuf_pool.tile([128, 2, 128], F32R, name="eb", tag="eb")
                e0 = ebuf_pool.tile([128, 2, 128], F32R, name="e0", tag="eb")
                bb = bb_pool.tile([32, 128], F32R, name="bb", tag="bb")
                nc.sync.dma_start(out=eb, in_=EMB_T[:, :, 128 * r : 128 * (r + 1)].rearrange("a p m -> p a m"))
                nc.sync.dma_start(out=e0, in_=EMB0_T[:, :, 128 * r : 128 * (r + 1)].rearrange("a p m -> p a m"))
                nc.sync.dma_start(out=bb, in_=BB[r])
                # ring tile is f32r-typed only for the benefit of the inject
                # matmul's *xstep* DMA chain; the DVE writes it via an f32
                # view (legal: the ring itself never feeds a matmul directly)
                xsr = xsel_pool.tile([128, GW], F32R, name="xsel", tag="xsel")
                for ch in range(4):
                    xb = acc_pool.tile([128, 512], F32, name="xb", tag="acc")
                    cols = slice(512 * ch, 512 * (ch + 1))
                    nc.tensor.matmul(xb, lhsT=eb[:, 0, :], rhs=rhs_e[:, 0, cols], start=True, stop=False)
                    nc.tensor.matmul(xb, lhsT=eb[:, 1, :], rhs=rhs_e[:, 1, cols], start=False, stop=False)
                    nc.tensor.matmul(xb, lhsT=e0[:, 0, :], rhs=rhs_e[:, 2, cols], start=False, stop=False)
                    nc.tensor.matmul(xb, lhsT=e0[:, 1, :], rhs=rhs_e[:, 3, cols], start=False, stop=False)
                    nc.tensor.matmul(xb, lhsT=bb, rhs=rhs_b[:, cols], start=False, stop=True)
                    nc.vector.tensor_copy(out=xsr.bitcast(F32)[:, cols], in_=xb)
                xsel_tiles[r] = xsr
                return xsr

            for r in range(min(XLEAD, R)):
                xsel_rowtile(r)

            # ---------------- LSTM loop ----------------
            for t in range(T):
                rt, mrow = divmod(t, SPT)  # xsel row tile, step-within-tile
                if t % SPT == 0 and rt + XLEAD < R:
                    xsel_rowtile(rt + XLEAD)
                xs = xsel_tiles[rt]

                sct = sc_pool.tile([128, 64], F32, name="sct", tag="sct")
                nc.sync.dma_start(out=sct, in_=SC[t])
                # this step's Xsel row block, moved to base partition 0 for
                # the inject matmul's moving operand
                xstep = xstep_pool.tile([B, GW], F32R, name="xstep", tag="xstep")
                nc.sync.dma_start(out=xstep, in_=xs[B * mrow : B * (mrow + 1), :])

                # inject Xsel (no h dependency -- can start early), then the
                # 8 recurrent K-tiles per gate strip
                for s in range(NG):
                    nc.tensor.matmul(
                        gates_ps[0:B, 512 * s : 512 * (s + 1)],
                        lhsT=i16r,
                        rhs=xstep[:, 512 * s : 512 * (s + 1)],
                        start=True, stop=False, skip_group_check=True,
                    )
                for k in range(2 * NJ):
                    for s in range(NG):
                        if k < NJ:
                            lhs = hsT[:, k, t, :]
                            rhs = rhs_h[:, k, 512 * s : 512 * (s + 1)]
                        else:
                            j = k - NJ
                            lhs = h0buf.bitcast(F32R)[:, B * j : B * (j + 1)]
                            rhs = rhs_h[:, NJ + j, 512 * s : 512 * (s + 1)]
                        nc.tensor.matmul(
                            gates_ps[0:B, 512 * s : 512 * (s + 1)],
                            lhsT=lhs, rhs=rhs,
                            start=False, stop=(k == 2 * NJ - 1),
                            skip_group_check=True,
                        )

                # gate order along the 2048 axis is [i,f,o,g] (host-packed):
                # tanh(x/2) for the sigmoid gates, tanh(x) for g
                nc.scalar.activation(
                    out=tg[:, 0:3 * H], in_=gates_ps[0:B, 0:3 * H],
                    func=mybir.ActivationFunctionType.Tanh, scale=0.5,
                )
                nc.scalar.activation(
                    out=tg[:, 3 * H:], in_=gates_ps[0:B, 3 * H:],
                    func=mybir.ActivationFunctionType.Tanh, scale=1.0,
                )

                # transpose activated gates: [16,128] -> [128,16] per (s,j)
                # tg strip order is [i,f,o,g]; tgps group order is [i,f,g,o]
                for s_tg, s_ps in ((0, 0), (1, 1), (2, 3), (3, 2)):
                    for j in range(NJ):
                        nc.tensor.transpose(
                            tgps[:, (s_ps * NJ + j) * B : (s_ps * NJ + j + 1) * B],
                            tg[:, 128 * (s_tg * NJ + j) : 128 * (s_tg * NJ + j) + 128],
                            idmat[0:B, :],
                        )

                gi = tgps[:, 0 * NJ * B : 1 * NJ * B]
                gf = tgps[:, 1 * NJ * B : 2 * NJ * B]
                gg = tgps[:, 2 * NJ * B : 3 * NJ * B]
                go = tgps[:, 3 * NJ * B : 4 * NJ * B]

                # u = (tf+1)*c' ; v = (ti+1)*g^ ; c' = 0.5u + v
                nc.vector.scalar_tensor_tensor(
                    out=tmp_u, in0=gf, scalar=1.0, in1=cprime,
                    op0=mybir.AluOpType.add, op1=mybir.AluOpType.mult,
                )
                # DVE reads only one PSUM operand per op: stage g^ via ACT
                nc.scalar.copy(out=gg_sb, in_=gg)
                nc.vector.scalar_tensor_tensor(
                    out=tmp_v, in0=gi, scalar=1.0, in1=gg_sb,
                    op0=mybir.AluOpType.add, op1=mybir.AluOpType.mult,
                )
                nc.vector.scalar_tensor_tensor(
                    out=cprime, in0=tmp_u, scalar=0.5, in1=tmp_v,
                    op0=mybir.AluOpType.mult, op1=mybir.AluOpType.add,
                )
                nc.scalar.activation(
                    out=tanh_c, in_=cprime,
                    func=mybir.ActivationFunctionType.Tanh, scale=0.5,
                )
                # h' = 2h = (to+1)*tanh(c)
                nc.vector.scalar_tensor_tensor(
                    out=hsT[:, :, t + 1, :],
                    in0=go.rearrange("p (a b) -> p a b", a=NJ),
                    scalar=1.0,
                    in1=tanh_c.rearrange("p (a b) -> p a b", a=NJ),
                    op0=mybir.AluOpType.add, op1=mybir.AluOpType.mult,
                )
                # h0 = h' * m0[t+1]  (mask columns pre-shifted on host)
                nc.gpsimd.tensor_tensor(
                    out=h0buf.rearrange("p (a b) -> p a b", a=NJ),
                    in0=_f(hsT[:, :, t + 1, :]),
                    in1=sct.rearrange("p (a b) -> p a b", a=NJ),
                    op=mybir.AluOpType.mult,
                )

            nc.sync.dma_start(
                out=HT2.rearrange("p (a b) -> p a b", a=NJ), in_=hsT[:, :, T, :]
            )
            nc.sync.dma_start(out=CT2, in_=cprime)

            # ---------------- phase 2 ----------------
            for n, (v0, cw) in enumerate(chunks):
                nc.sync.dma_start(
                    out=wout_sb[:, :, v0 : v0 + cw], in_=WOUT[:, :, v0 : v0 + cw]
                )
            if has_bout:
                nc.sync.dma_start(out=bout_sb, in_=BOUT)

            def row_lhs(r):
                """stationary [128 K, 128 rows] accessor for t-major row-tile r:
                row m of tile r is (t = (128r+m)//16, b = m%16) ->
                hsT[:, k, t+1, b]; free stride over m is 1 in (t,b) raster ==
                exactly the hsT (tau, b) layout."""
                t0 = (128 * r) // B
                return lambda k: hsT[:, k, t0 + 1 : t0 + 1 + SPT, :].rearrange(
                    "p t b -> p (t b)"
                )

            # pass 1: s_partial[r] = sum_v exp(z + bout)
            for r in range(R):
                rl = row_lhs(r)
                for n, (v0, cw) in enumerate(chunks):
                    zps = acc_pool.tile([128, 512], F32, name="zps", tag="acc")[:, :cw]
                    for k in range(NJ):
                        nc.tensor.matmul(
                            zps, lhsT=rl(k), rhs=wout_sb[:, k, v0 : v0 + cw],
                            start=(k == 0), stop=(k == NJ - 1),
                        )
                    if has_bout:
                        scr = scr_pool.tile([128, chunk], F32, name="scr", tag="scr")[:, :cw]
                        nc.vector.tensor_tensor(
                            out=scr, in0=zps, in1=bout_sb[:, v0 : v0 + cw],
                            op=mybir.AluOpType.add,
                        )
                        ein = eout = scr
                    else:
                        ein = eout = zps  # in-place exp in PSUM; z not needed after
                    nc.scalar.activation(
                        out=eout, in_=ein, func=mybir.ActivationFunctionType.Exp,
                        accum_out=sparts[:, r, n : n + 1],
                    )

            # combine stats across cores
            nc.vector.reduce_sum(s_loc, sparts, axis=mybir.AxisListType.X)
            nc.sync.dma_start(out=cc_in, in_=s_loc)
            nc.gpsimd.collective_compute(
                "AllGather",
                mybir.AluOpType.bypass,
                replica_groups=[list(range(n_cores))],
                ins=[cc_in.opt()],
                outs=[cc_out.opt()],
            )
            nc.sync.dma_start(out=s_all, in_=cc_out.rearrange("g p r -> p r g"))
            nc.vector.reduce_sum(lse, s_all, axis=mybir.AxisListType.X)
            nc.scalar.activation(
                out=lse, in_=lse, func=mybir.ActivationFunctionType.Ln
            )

            # pass 2: out = z - lse (+ bout)
            for r in range(R):
                rl = row_lhs(r)
                for n, (v0, cw) in enumerate(chunks):
                    zps = acc_pool.tile([128, 512], F32, name="zps2", tag="acc")[:, :cw]
                    for k in range(NJ):
                        nc.tensor.matmul(
                            zps, lhsT=rl(k), rhs=wout_sb[:, k, v0 : v0 + cw],
                            start=(k == 0), stop=(k == NJ - 1),
                        )
                    ob = outb_pool.tile([128, chunk], F32, name="ob", tag="ob")[:, :cw]
                    if has_bout:
                        nc.vector.scalar_tensor_tensor(
                            out=ob, in0=zps, scalar=lse[:, r : r + 1],
                            in1=bout_sb[:, v0 : v0 + cw],
                            op0=mybir.AluOpType.subtract, op1=mybir.AluOpType.add,
                        )
                    else:
                        nc.vector.tensor_scalar_sub(ob, zps, lse[:, r : r + 1])
                    nc.sync.dma_start(
                        out=OUT[128 * r : 128 * (r + 1), v0 : v0 + cw], in_=ob
                    )


def prepare_inputs(x, assignments, init_h, init_c, embed_table, W_ih, W_hh, b,
                   W_out, b_out, T, V, n_cores):
    """Host-side layout preparation.  Returns (shared_map, per_core_maps, has_bout)."""
    x = np.asarray(x)
    assignments = np.asarray(assignments)
    f32 = np.float32
    init_h = np.asarray(init_h, f32)
    init_c = np.asarray(init_c, f32)
    embed_table = np.asarray(embed_table, f32)
    W_ih = np.asarray(W_ih, f32)
    W_hh = np.asarray(W_hh, f32)
    b = np.asarray(b, f32)
    W_out = np.asarray(W_out, f32)
    b_out = np.asarray(b_out, f32)
    ROWS = B * T
    R = ROWS // 128

    cid = assignments[x]  # [B, T]
    m0 = (cid == 0).astype(f32)  # [B, T]
    emb = embed_table[x]  # [B, T, E]

    # masks, shifted by one step (used at end of step t for step t+1)
    SCa = np.zeros((T, 128, 64), f32)
    m0s = np.concatenate([m0[:, 1:], np.zeros((B, 1), f32)], axis=1)
    SCa[:, :, :] = np.broadcast_to(
        m0s.T[:, None, None, :], (T, 128, 4, B)
    ).reshape(T, 128, 64)

    # t-major transposed embeddings: EMB_T[j, p, 16t+b] = emb[b, t, 128j+p]
    eT = np.transpose(emb.reshape(B, T, 2, 128), (2, 3, 1, 0))  # [j,p,t,b]
    EMB_Ta = np.ascontiguousarray(eT.reshape(2, 128, ROWS), f32)
    EMB0_Ta = np.ascontiguousarray(
        (eT * m0.T[None, None, :, :]).reshape(2, 128, ROWS), f32
    )

    # bias inject lhsT per row tile: rows [ones; m0(row)]
    BBa = np.zeros((R, 32, 128), f32)
    m0_rows = m0.T.reshape(ROWS)  # t-major
    BBa[:, 0, :] = 1.0
    BBa[:, 1, :] = m0_rows.reshape(R, 128)

    # device packs gate blocks [i, f, o, g] along the 2048 axis (so the
    # sigmoid-scaled tanh covers one contiguous range)
    gperm = np.concatenate([np.arange(0, H), np.arange(H, 2 * H),
                            np.arange(3 * H, 4 * H), np.arange(2 * H, 3 * H)])
    W_ih = W_ih[:, gperm, :]
    W_hh = W_hh[:, gperm, :]
    b = b[:, gperm]

    W1h = W_hh[1]
    dWh = (W_hh[0] - W_hh[1]).astype(f32)
    RHS_Ha = np.zeros((128, 2 * NJ, GW), f32)
    RHS_Ha[:, 0:NJ, :] = 0.5 * np.transpose(W1h.T.reshape(NJ, 128, GW), (1, 0, 2))
    RHS_Ha[:, NJ:, :] = 0.5 * np.transpose(dWh.T.reshape(NJ, 128, GW), (1, 0, 2))

    W1i = W_ih[1]
    dWi = (W_ih[0] - W_ih[1]).astype(f32)
    RHS_Ea = np.zeros((128, 4, GW), f32)
    RHS_Ea[:, 0:2, :] = np.transpose(W1i.T.reshape(2, 128, GW), (1, 0, 2))
    RHS_Ea[:, 2:4, :] = np.transpose(dWi.T.reshape(2, 128, GW), (1, 0, 2))

    RHS_Ba = np.zeros((32, GW), f32)
    RHS_Ba[0] = b[1]
    RHS_Ba[1] = b[0] - b[1]

    CONSTSa = np.zeros((128, 209), f32)
    i2h = 2.0 * np.transpose(init_h.reshape(B, NJ, 128), (2, 1, 0))  # [p,j,b]
    CONSTSa[:, 0:64] = i2h.reshape(128, 64)
    CONSTSa[:, 64:128] = (i2h * m0[None, None, :, 0]).reshape(128, 64)
    CONSTSa[:, 128:192] = (
        2.0 * np.transpose(init_c.reshape(B, NJ, 128), (2, 1, 0))
    ).reshape(128, 64)
    idm = np.zeros((128, 16), f32)
    idm[0:16, :] = np.eye(16, dtype=f32)
    CONSTSa[:, 192:208] = idm
    CONSTSa[:, 208] = 1.0  # scale for the g (tanh) strip rows 16:32 view
    I16Ra = np.eye(16, dtype=f32)

    has_bout = bool(np.any(b_out != 0.0))

    shared = dict(
        SC=SCa, RHS_H=RHS_Ha, RHS_E=RHS_Ea, RHS_B=RHS_Ba, EMB_T=EMB_Ta,
        EMB0_T=EMB0_Ta, BB=BBa, CONSTS=CONSTSa, I16R=I16Ra,
    )

    VS = V // n_cores
    per_core = []
    for c in range(n_cores):
        w = W_out[c * VS : (c + 1) * VS]  # [VS, H]
        WOUTa = 0.5 * np.transpose(w.T.reshape(NJ, 128, VS), (1, 0, 2))
        m = dict(WOUT=np.ascontiguousarray(WOUTa, f32))
        if has_bout:
            m["BOUT"] = np.ascontiguousarray(
                np.broadcast_to(b_out[c * VS : (c + 1) * VS], (128, VS)), f32
            )
        per_core.append(m)
    return shared, per_core, has_bout


def assemble_outputs(results, T, V, n_cores):
    # logits rows are t-major (row = 16t + b); reference wants b-major
    lg = np.concatenate([r["OUT"] for r in results], axis=1)
    lg = lg.reshape(T, B, V).transpose(1, 0, 2).reshape(B * T, V)
    HT2 = results[0]["HT2"]
    CT2 = results[0]["CT2"]
    hT = 0.5 * np.transpose(HT2.reshape(128, NJ, B), (2, 1, 0)).reshape(B, H)
    cT = 0.5 * np.transpose(CT2.reshape(128, NJ, B), (2, 1, 0)).reshape(B, H)
    return (
        np.ascontiguousarray(lg, np.float32),
        hT.astype(np.float32),
        cT.astype(np.float32),
    )


def run(inputs, T=256, V=32000, n_cores=8, trace=False):
    shared, per_core, has_bout = prepare_inputs(
        T=T, V=V, n_cores=n_cores, **inputs
    )
    nc = bacc.Bacc(
        "TRN2", target_bir_lowering=False, debug=False, num_devices=n_cores
    )
    build_graph(nc, T=T, V=V, n_cores=n_cores, has_bout=has_bout)
    nc.compile()
    in_maps = [dict(shared, **pc) for pc in per_core]
    res = run_bass_kernel_spmd(
        nc, in_maps, core_ids=list(range(n_cores)), trace=trace
    )
    outs = assemble_outputs(res.results, T, V, n_cores)
    return outs, res


def kernel(**inputs):
    (logits, hT, cT), _ = run(inputs)
    return logits, hT, cT
